# revision 7
# baseline (speedup 1.0000x reference)
"""Trainium2 (8 NeuronCores) kernel for a gated-attention transformer block.

Reference computation (per batch b):
    q = x@Wq, [k|v] = x@Wkv, heads=8, dh=64
    attn = softmax(q k^T / 8) v
    out  = (attn * sigmoid(x@Wg + bg)) @ Wo + bo + x
    out  = LayerNorm(out) * gamma + beta

Sharding: 8 cores = 4 batches x 2 sequence-halves. Each core computes
k/v for its full batch (duplicated across the half-pair; avoids any
collective) and q/gates/output for its own 1024 rows. Row order of
keys/values is irrelevant to attention, so each core receives x[b]
rolled so its own rows come first; compile-time indices are then
identical across cores (SPMD-safe).

On-chip layout: activations transposed ([feature, seq]) for projections
and attention; dots computed as dotsT[j, i] with 2x row-tiled matmuls
(K=64 head pairs on PE quadrants), softmax denominator via a ones-column
augmented attn@v matmul (M=65), gating + denominator applied in
transposed layout, final Wo projection back to natural layout for the
residual + LayerNorm tail. All matmuls bf16 with fp32 PSUM accumulation.
"""

import sys
import os
import numpy as np

for _p in ("/opt/trn_rl_repo", "/root/.axon_site/_ro/trn_rl_repo"):
    if os.path.isdir(_p) and _p not in sys.path:
        sys.path.insert(0, _p)

import concourse.bass as bass
import concourse.tile as tile
from concourse import bacc, mybir
from concourse.bass_utils import run_bass_kernel_spmd
from concourse.masks import make_identity

F32 = mybir.dt.float32
BF16 = mybir.dt.bfloat16
AF = mybir.ActivationFunctionType
OP = mybir.AluOpType

B, N, D, H, DH = 4, 2048, 512, 8, 64
NH = N // 2          # rows owned per core
SCALE = DH ** -0.5   # 0.125
EPS = 1e-5
NCORES = 8


def build_nc():
    nc = bacc.Bacc("TRN2", target_bir_lowering=False, debug=False,
                   num_devices=NCORES)

    xkv = nc.dram_tensor("xkv", [N, D], F32, kind="ExternalInput")
    Wq = nc.dram_tensor("Wq", [D, D], F32, kind="ExternalInput")
    Wk = nc.dram_tensor("Wk", [D, D], F32, kind="ExternalInput")
    Wv = nc.dram_tensor("Wv", [D, D], F32, kind="ExternalInput")
    Wg = nc.dram_tensor("Wg", [D, D], F32, kind="ExternalInput")
    Wo = nc.dram_tensor("Wo", [D, D], F32, kind="ExternalInput")
    bg = nc.dram_tensor("bg", [D], F32, kind="ExternalInput")
    bo = nc.dram_tensor("bo", [D], F32, kind="ExternalInput")
    gamma = nc.dram_tensor("gamma", [D], F32, kind="ExternalInput")
    beta = nc.dram_tensor("beta", [D], F32, kind="ExternalInput")
    out = nc.dram_tensor("out", [NH, D], F32, kind="ExternalOutput")

    def bcast_ap(t, n):
        return bass.AP(tensor=t, offset=0, ap=[[0, 128], [1, n]])

    with tile.TileContext(nc) as tc:
        with tc.tile_pool(name="consts", bufs=1) as consts, \
             tc.tile_pool(name="wpool", bufs=1) as wpool, \
             tc.tile_pool(name="acts", bufs=1) as acts, \
             tc.tile_pool(name="stage", bufs=2) as stage, \
             tc.tile_pool(name="prpool", bufs=4) as prpool, \
             tc.tile_pool(name="ppool", bufs=2, space="PSUM") as ppool, \
             tc.tile_pool(name="papool", bufs=2, space="PSUM") as papool, \
             tc.tile_pool(name="pmisc", bufs=2, space="PSUM") as pmisc:

            # ---- constants ----
            ident = consts.tile([128, 128], BF16)
            make_identity(nc, ident[:])
            bg_t = consts.tile([64, H], F32)
            nc.sync.dma_start(bg_t[:], bg.ap().rearrange("(h p) -> p h", p=64))
            bo_b = consts.tile([128, D], F32)
            nc.sync.dma_start(bo_b[:], bcast_ap(bo, D))
            gam_b = consts.tile([128, D], F32)
            nc.sync.dma_start(gam_b[:], bcast_ap(gamma, D))
            bet_b = consts.tile([128, D], F32)
            nc.sync.dma_start(bet_b[:], bcast_ap(beta, D))
            eps_t = consts.tile([128, 1], F32)
            nc.vector.memset(eps_t[:], EPS)

            # ---- weights: load fp32, cast to bf16 ----
            w_bf = {}
            for name, t in (("Wq", Wq), ("Wk", Wk), ("Wv", Wv), ("Wg", Wg)):
                ws = stage.tile([128, 4, D], F32, tag="wstage", bufs=1)
                nc.sync.dma_start(ws[:], t.ap().rearrange("(c p) n -> p c n", p=128))
                wb = wpool.tile([128, 4, D], BF16)
                nc.vector.tensor_copy(wb[:], ws[:])
                w_bf[name] = wb
            wos = stage.tile([64, H, D], F32, tag="wostage", bufs=1)
            nc.sync.dma_start(wos[:], Wo.ap().rearrange("(h p) n -> p h n", p=64))
            wo_b = wpool.tile([64, H, D], BF16)
            nc.vector.tensor_copy(wo_b[:], wos[:])

            # ---- x: load, cast bf16, transpose to xT [128,4(Dc),N] ----
            xT = acts.tile([128, 4, N], BF16)
            for nt in range(N // 128):
                xs = stage.tile([128, D], F32, tag="xstage")
                nc.sync.dma_start(xs[:], xkv[nt * 128:(nt + 1) * 128, :])
                xb = stage.tile([128, D], BF16, tag="xbf")
                nc.vector.tensor_copy(xb[:], xs[:])
                for kc in range(4):
                    pt = pmisc.tile([128, 128], BF16, tag="m")
                    nc.tensor.transpose(pt[:], xb[:, kc * 128:(kc + 1) * 128], ident[:])
                    nc.vector.tensor_copy(xT[:, kc, nt * 128:(nt + 1) * 128], pt[:])

            # ---- projections ----
            # qT [128,4(pair),NH]: pair m holds heads (2m, 2m+1)
            qT = acts.tile([128, 4, NH], BF16)
            for m in range(4):
                for ic in range(NH // 512):
                    pm = pmisc.tile([128, 512], F32, tag="m")
                    for kc in range(4):
                        nc.tensor.matmul(pm[:], w_bf["Wq"][:, kc, m * 128:(m + 1) * 128],
                                         xT[:, kc, ic * 512:(ic + 1) * 512],
                                         start=(kc == 0), stop=(kc == 3))
                    nc.vector.tensor_copy(qT[:, m, ic * 512:(ic + 1) * 512], pm[:])
            # kT [128,4(pair),N]
            kT = acts.tile([128, 4, N], BF16)
            for m in range(4):
                for ic in range(N // 512):
                    pm = pmisc.tile([128, 512], F32, tag="m")
                    for kc in range(4):
                        nc.tensor.matmul(pm[:], w_bf["Wk"][:, kc, m * 128:(m + 1) * 128],
                                         xT[:, kc, ic * 512:(ic + 1) * 512],
                                         start=(kc == 0), stop=(kc == 3))
                    nc.vector.tensor_copy(kT[:, m, ic * 512:(ic + 1) * 512], pm[:])
            # v natural with ones column: v3 [128(j),16(jt),H,65]
            v3 = acts.tile([128, N // 128, H, DH + 1], BF16)
            nc.vector.memset(v3[:, :, :, DH:DH + 1], 1.0)
            for jt in range(N // 128):
                pm = pmisc.tile([128, 512], F32, tag="m")
                for kc in range(4):
                    nc.tensor.matmul(pm[:], xT[:, kc, jt * 128:(jt + 1) * 128],
                                     w_bf["Wv"][:, kc, :],
                                     start=(kc == 0), stop=(kc == 3))
                nc.vector.tensor_copy(
                    v3[:, jt, :, 0:DH],
                    pm[:].rearrange("p (h d) -> p h d", h=H))
            # gates -> sigmoid, per-head sigT [64,H,NH]
            sigT = acts.tile([64, H, NH], BF16)
            for m in range(4):
                for ic in range(NH // 512):
                    pm = pmisc.tile([128, 512], F32, tag="m")
                    for kc in range(4):
                        nc.tensor.matmul(pm[:], w_bf["Wg"][:, kc, m * 128:(m + 1) * 128],
                                         xT[:, kc, ic * 512:(ic + 1) * 512],
                                         start=(kc == 0), stop=(kc == 3))
                    nc.scalar.activation(sigT[:, 2 * m, ic * 512:(ic + 1) * 512],
                                         pm[0:64, :], AF.Sigmoid,
                                         bias=bg_t[:, 2 * m:2 * m + 1])
                    nc.scalar.activation(sigT[:, 2 * m + 1, ic * 512:(ic + 1) * 512],
                                         pm[64:128, :], AF.Sigmoid,
                                         bias=bg_t[:, 2 * m + 1:2 * m + 2])

            # ---- attention, per head pair ----
            gatedT = acts.tile([64, H, NH], BF16)
            for p in range(4):
                for ic in range(NH // 512):
                    pe_ = papool.tile([128, 512], F32, tag="att")
                    po_ = papool.tile([128, 512], F32, tag="att")
                    for jt in range(N // 128):
                        pd = ppool.tile([128, 1024], F32)
                        nc.tensor.matmul(pd[:, 0:512],
                                         kT[0:64, p, jt * 128:(jt + 1) * 128],
                                         qT[0:64, p, ic * 512:(ic + 1) * 512],
                                         start=True, stop=True,
                                         tile_position=(0, 0))
                        nc.tensor.matmul(pd[:, 512:1024],
                                         kT[64:128, p, jt * 128:(jt + 1) * 128],
                                         qT[64:128, p, ic * 512:(ic + 1) * 512],
                                         start=True, stop=True,
                                         tile_position=(64, 0))
                        # exp(scale * dots) for both heads in one ACT op
                        pr = prpool.tile([128, 2, 512], BF16, tag="pr")
                        nc.scalar.activation(
                            pr[:], pd[:].rearrange("p (h x) -> p h x", h=2),
                            AF.Exp, scale=SCALE)
                        nc.tensor.matmul(pe_[0:65, :], v3[:, jt, 2 * p, :],
                                         pr[:, 0, :],
                                         start=(jt == 0), stop=(jt == N // 128 - 1))
                        nc.tensor.matmul(po_[0:65, :], v3[:, jt, 2 * p + 1, :],
                                         pr[:, 1, :],
                                         start=(jt == 0), stop=(jt == N // 128 - 1))
                    for hh, ph in ((2 * p, pe_), (2 * p + 1, po_)):
                        r0 = stage.tile([1, 512], F32, tag="r0")
                        nc.vector.reciprocal(r0[:], ph[64:65, :])
                        rb = stage.tile([64, 512], F32, tag="rb")
                        nc.gpsimd.partition_broadcast(rb[:], r0[:])
                        tmp = stage.tile([64, 512], F32, tag="tmp")
                        nc.vector.tensor_mul(tmp[:], ph[0:64, :], rb[:])
                        nc.vector.tensor_mul(gatedT[:, hh, ic * 512:(ic + 1) * 512],
                                             tmp[:], sigT[:, hh, ic * 512:(ic + 1) * 512])

            # ---- Wo projection (natural out) + residual + LayerNorm ----
            for it in range(NH // 128):
                pw = pmisc.tile([128, 512], F32, tag="m")
                for h in range(H):
                    nc.tensor.matmul(pw[:], gatedT[:, h, it * 128:(it + 1) * 128],
                                     wo_b[:, h, :], start=(h == 0), stop=(h == H - 1))
                xres = stage.tile([128, D], F32, tag="xres")
                nc.sync.dma_start(xres[:], xkv[it * 128:(it + 1) * 128, :])
                y = stage.tile([128, D], F32, tag="y")
                nc.vector.tensor_add(y[:], pw[:], xres[:])
                nc.vector.tensor_add(y[:], y[:], bo_b[:])
                st = stage.tile([128, 6], F32, tag="st")
                nc.vector.bn_stats(st[:], y[:])
                mv = stage.tile([128, 2], F32, tag="mv")
                nc.vector.bn_aggr(mv[:], st[:])
                ve = stage.tile([128, 1], F32, tag="ve")
                nc.vector.tensor_add(ve[:], mv[:, 1:2], eps_t[:])
                nc.vector.reciprocal(ve[:], ve[:])
                nc.scalar.activation(ve[:], ve[:], AF.Sqrt)
                z = stage.tile([128, D], F32, tag="z")
                nc.vector.tensor_scalar(z[:], y[:], mv[:, 0:1], ve[:],
                                        OP.subtract, OP.mult)
                nc.vector.tensor_mul(z[:], z[:], gam_b[:])
                nc.vector.tensor_add(z[:], z[:], bet_b[:])
                nc.sync.dma_start(out[it * 128:(it + 1) * 128, :], z[:])

    nc.compile()
    return nc


_NC_CACHE = None


def _get_nc():
    global _NC_CACHE
    if _NC_CACHE is None:
        _NC_CACHE = build_nc()
    return _NC_CACHE


def kernel(**inputs) -> np.ndarray:
    x = np.asarray(inputs["x"], dtype=np.float32)
    Wq = np.ascontiguousarray(np.asarray(inputs["Wq"], dtype=np.float32))
    Wkv = np.asarray(inputs["Wkv"], dtype=np.float32)
    Wk = np.ascontiguousarray(Wkv[:, :D])
    Wv = np.ascontiguousarray(Wkv[:, D:])
    Wg = np.ascontiguousarray(np.asarray(inputs["Wg"], dtype=np.float32))
    Wo = np.ascontiguousarray(np.asarray(inputs["Wo"], dtype=np.float32))
    bg = np.ascontiguousarray(np.asarray(inputs["bg"], dtype=np.float32))
    bo = np.ascontiguousarray(np.asarray(inputs["bo"], dtype=np.float32))
    gamma = np.ascontiguousarray(np.asarray(inputs["gamma"], dtype=np.float32))
    beta = np.ascontiguousarray(np.asarray(inputs["beta"], dtype=np.float32))

    nc = _get_nc()
    in_maps = []
    for c in range(NCORES):
        b, half = c // 2, c % 2
        rolled = np.ascontiguousarray(np.roll(x[b], -half * NH, axis=0))
        in_maps.append({"xkv": rolled, "Wq": Wq, "Wk": Wk, "Wv": Wv,
                        "Wg": Wg, "Wo": Wo, "bg": bg, "bo": bo,
                        "gamma": gamma, "beta": beta})
    res = run_bass_kernel_spmd(nc, in_maps, core_ids=list(range(NCORES)))
    out = np.empty((B, N, D), dtype=np.float32)
    for c in range(NCORES):
        b, half = c // 2, c % 2
        out[b, half * NH:(half + 1) * NH] = res.results[c]["out"]
    return out
